# revision 1
# baseline (speedup 1.0000x reference)
"""FEDformer encoder layer on 8 TRN2 NeuronCores — batch-data-parallel Bass kernel.

Strategy (self-contained; shapes hardcoded):
  B=16,L=2048,D=512,H=8,E=64,M=64,DFF=2048; 8 cores x 2 batches each; no collectives.

  Math restructuring (validated against the jax reference):
   - rfft+mode-gather == x @ Fcat where Fcat[l, 0:64]=cos(2*pi*k_j*l/L),
     Fcat[l, 64:128]=-sin(...), k_j = mode_index.
   - The q-projection (Wq) and out-projection (Wo) commute with the DFT, so they
     are applied in mode space ([128 x 512] instead of [2048 x 512] per batch;
     16x cheaper). k/v projections are dead code in the reference.
   - irfft of a spectrum with only bins 0..63 populated == P @ C2S2 where
     C2S2[0:64, t]=w_m cos(2*pi*m*t/L), C2S2[64:128, t]=-w_m sin(...),
     w_0=1/L, w_m=2/L  (Im of bin 0 drops automatically since sin(0)=0).
   - Fourier branch contributes ~1e-5 absolute to an O(1) output -> bf16 there.
   - series-decomp: K=2 softmax == sigmoid of weight/bias deltas; moving
     averages via one fp32 cumsum (tensor_tensor_scan) + shifted subtracts,
     replicate-pad handled by exact edge-correction terms.
   - FFN (the FLOP bulk) in float32r (measured 1.5e-4 relative on HW, 4x
     faster than fp32); gelu (exact erf form) via ACT Gelu LUT (2e-6 abs).

  Layout: device works feature-major ([D, L]); the host transposes x in and the
  output back during shard/unshard.
"""

import numpy as np

B, L, D, H, M, DFF = 16, 2048, 512, 8, 64, 2048
E = D // H
NC_ = 8
BLOC = B // NC_          # batches per core
MEXT = 2 * M             # re|im rows
NDC = D // 128           # 4 feature tiles
NFF = DFF // 128         # 16 dff tiles
NLC = L // 128           # 16 token chunks of 128
NTC = L // 512           # 4 token chunks of 512

_prog_cache = {}
_fixn = [0]


def _fix_sync_waits(nc, max_waits=1, max_updates=4):
    """Split >max sem-waits/updates per instruction onto adjacent nops.

    The AWS neuronx-cc walrus rejects instructions carrying too many sync
    commands ("Too many sync wait commands"); Tile's tail drain aggregates one
    wait per outstanding semaphore. Engine-order execution makes the split
    semantically identical.
    """
    import concourse.mybir as mybir

    for f in nc.m.functions:
        for bb in f.blocks:
            insts = bb.instructions
            i = 0
            while i < len(insts):
                ins = insts[i]
                si = ins.sync_info
                if si is not None and si.on_wait and len(si.on_wait) > max_waits:
                    waits = list(si.on_wait)
                    si.on_wait = waits[-max_waits:]
                    rest = waits[:-max_waits]
                    chunks = [rest[j:j + max_waits]
                              for j in range(0, len(rest), max_waits)]
                    for c in reversed(chunks):
                        _fixn[0] += 1
                        nop = mybir.InstNoOp(name=f"I-fixw-{_fixn[0]}", ins=[], outs=[])
                        nop.engine = ins.engine
                        nop.sync_info = mybir.SyncInfo(on_wait=c, on_update=[])
                        insts.insert(i, nop)
                        i += 1
                if si is not None and si.on_update and len(si.on_update) > max_updates:
                    ups = list(si.on_update)
                    si.on_update = ups[:max_updates]
                    rest = ups[max_updates:]
                    chunks = [rest[j:j + max_updates]
                              for j in range(0, len(rest), max_updates)]
                    for c in chunks:
                        _fixn[0] += 1
                        nop = mybir.InstNoOp(name=f"I-fixu-{_fixn[0]}", ins=[], outs=[])
                        nop.engine = ins.engine
                        nop.sync_info = mybir.SyncInfo(on_wait=[], on_update=c)
                        insts.insert(i + 1, nop)
                        i += 1
                i += 1


def _build_program(need_bq, j0, fix=True):
    import concourse.bass as bass
    import concourse.mybir as mybir
    from concourse.tile import TileContext

    F32 = mybir.dt.float32
    F32R = mybir.dt.float32r
    BF16 = mybir.dt.bfloat16
    AF = mybir.ActivationFunctionType
    OP = mybir.AluOpType

    nc = bass.Bass()

    # ---- DRAM I/O ----
    XT = nc.dram_tensor("XT", [BLOC, D, L], F32, kind="ExternalInput")
    XBF = nc.dram_tensor("XBF", [BLOC, 128, NLC * D], BF16, kind="ExternalInput")
    FCT = nc.dram_tensor("FCT", [128, NLC * 128], BF16, kind="ExternalInput")
    C2S2 = nc.dram_tensor("C2S2", [128, L], BF16, kind="ExternalInput")
    WQT = nc.dram_tensor("WQT", [D, D], BF16, kind="ExternalInput")
    WOT = nc.dram_tensor("WOT", [D, D], BF16, kind="ExternalInput")
    WPK = nc.dram_tensor("WPK", [H, 128, M * 128], BF16, kind="ExternalInput")
    W1T = nc.dram_tensor("W1T", [D, DFF], F32R, kind="ExternalInput")
    W2T = nc.dram_tensor("W2T", [DFF, D], F32R, kind="ExternalInput")
    EYE = nc.dram_tensor("EYE", [128, 128], BF16, kind="ExternalInput")
    BO4 = nc.dram_tensor("BO4", [128, NDC], F32, kind="ExternalInput")
    BQ4 = nc.dram_tensor("BQ4", [128, NDC], F32, kind="ExternalInput")
    ECH13 = nc.dram_tensor("ECH13", [128, 7], F32, kind="ExternalInput")
    ETL13 = nc.dram_tensor("ETL13", [128, 6], F32, kind="ExternalInput")
    ECH25 = nc.dram_tensor("ECH25", [128, 13], F32, kind="ExternalInput")
    ETL25 = nc.dram_tensor("ETL25", [128, 12], F32, kind="ExternalInput")
    DECS = nc.dram_tensor("DECS", [128, 4], F32, kind="ExternalInput")
    OUT_T = nc.dram_tensor("OUT_T", [BLOC, D, L], F32, kind="ExternalOutput")

    with TileContext(nc) as tc:
        # ---------- persistent pools (LIFO close at the end) ----------
        cst = tc.tile_pool(name="cst", bufs=1)
        cstp = cst.__enter__()
        main = tc.tile_pool(name="main", bufs=1)
        mainp = main.__enter__()

        fct = cstp.tile([128, NLC * 128], BF16, name="fct")
        nc.sync.dma_start(out=fct[:], in_=FCT[:])
        c2s2 = cstp.tile([128, L], BF16, name="c2s2")
        nc.sync.dma_start(out=c2s2[:], in_=C2S2[:])
        wqt = [cstp.tile([128, D], BF16, name=f"wqt{i}") for i in range(NDC)]
        wot = [cstp.tile([128, D], BF16, name=f"wot{i}") for i in range(NDC)]
        for i in range(NDC):
            nc.sync.dma_start(out=wqt[i][:], in_=WQT[i * 128:(i + 1) * 128, :])
            nc.sync.dma_start(out=wot[i][:], in_=WOT[i * 128:(i + 1) * 128, :])
        eye = cstp.tile([128, 128], BF16, name="eye")
        nc.sync.dma_start(out=eye[:], in_=EYE[:])
        bo4 = cstp.tile([128, NDC], F32, name="bo4")
        nc.sync.dma_start(out=bo4[:], in_=BO4[:])
        ech13 = cstp.tile([128, 7], F32, name="ech13")
        etl13 = cstp.tile([128, 6], F32, name="etl13")
        ech25 = cstp.tile([128, 13], F32, name="ech25")
        etl25 = cstp.tile([128, 12], F32, name="etl25")
        decs = cstp.tile([128, 4], F32, name="decs")
        for t_, src in ((ech13, ECH13), (etl13, ETL13), (ech25, ECH25),
                        (etl25, ETL25), (decs, DECS)):
            nc.sync.dma_start(out=t_[:], in_=src[:])
        bq4 = None
        if need_bq:
            bq4 = cstp.tile([128, NDC], F32, name="bq4")
            nc.sync.dma_start(out=bq4[:], in_=BQ4[:])

        # main activation buffer: xT -> u -> r1 -> v -> out^T, all in place
        decp_cm = tc.tile_pool(name="decp", bufs=1)
        decp = decp_cm.__enter__()
        mt = [[mainp.tile([128, L], F32, name=f"m_{b}_{dc}") for dc in range(NDC)]
              for b in range(BLOC)]
        for b in range(BLOC):
            for dc in range(NDC):
                nc.sync.dma_start(out=mt[b][dc][:],
                                  in_=XT[b, dc * 128:(dc + 1) * 128, :])

        # ---------- series decomposition ----------
        def decomp(dec_pool, b, dc, dw_col, db_col):
            """mt[b][dc] (fp32 [128, L]) -> series-decomp residual, in place."""
            u = mt[b][dc]
            cs = dec_pool.tile([128, L], F32, name="cs", tag="cs")
            s13 = dec_pool.tile([128, L], F32, name="s13", tag="s13")
            s25 = dec_pool.tile([128, L], F32, name="s25", tag="s25")
            g = s25  # gate reuses s25's storage once the scaled copy lands in cs
            sm = dec_pool.tile([128, 40], F32, name="sm", tag="sm")
            # inclusive cumsum along tokens
            nc.vector.tensor_tensor_scan(cs[:], u[:], u[:], 0.0, OP.add, OP.bypass)
            # S13 = 13-window replicate-padded sums
            nc.vector.tensor_tensor(s13[:, 7:2042], cs[:, 13:2048], cs[:, 0:2035],
                                    OP.subtract)
            nc.vector.tensor_scalar_mul(sm[:, 0:7], ech13[:], u[:, 0:1])
            nc.vector.tensor_tensor(s13[:, 0:7], cs[:, 6:13], sm[:, 0:7], OP.add)
            nc.vector.tensor_scalar_mul(sm[:, 7:13], etl13[:], u[:, 2047:2048])
            nc.vector.scalar_tensor_tensor(
                s13[:, 2042:2048], sm[:, 7:13], cs[:, 2047:2048],
                cs[:, 2035:2041], OP.add, OP.subtract)
            # S25 on gpsimd
            nc.gpsimd.tensor_tensor(s25[:, 13:2036], cs[:, 25:2048], cs[:, 0:2023],
                                    OP.subtract)
            nc.vector.tensor_scalar_mul(sm[:, 13:26], ech25[:], u[:, 0:1])
            nc.gpsimd.tensor_tensor(s25[:, 0:13], cs[:, 12:25], sm[:, 13:26], OP.add)
            nc.vector.tensor_scalar_mul(sm[:, 26:38], etl25[:], u[:, 2047:2048])
            nc.vector.scalar_tensor_tensor(
                s25[:, 2036:2048], sm[:, 26:38], cs[:, 2047:2048],
                cs[:, 2023:2035], OP.add, OP.subtract)
            # ma25 = S25/25 (into cs, which is dead now); delta = S13/13 - ma25
            nc.vector.tensor_scalar_mul(cs[:], s25[:], 1.0 / 25.0)
            # gate = sigmoid(dw*u + db)  (overwrites s25)
            nc.scalar.activation(g[:], u[:], AF.Sigmoid,
                                 scale=decs[:, dw_col:dw_col + 1],
                                 bias=decs[:, db_col:db_col + 1])
            nc.vector.scalar_tensor_tensor(
                s13[:], s13[:], 1.0 / 13.0, cs[:], OP.mult, OP.subtract)
            # e = g*delta ; f = u - ma25 ; r = f - e -> u
            nc.gpsimd.tensor_tensor(g[:], g[:], s13[:], OP.mult)
            nc.gpsimd.tensor_tensor(cs[:], u[:], cs[:], OP.subtract)
            nc.vector.tensor_tensor(u[:], cs[:], g[:], OP.subtract)

        # FFN weights arrive during decomp1 (DMA overlaps DVE/Pool work)
        ffnw = tc.tile_pool(name="ffnw", bufs=1)
        ffnwp = ffnw.__enter__()
        w1t = [ffnwp.tile([128, DFF], F32R, name=f"w1t{i}") for i in range(NDC)]
        for i in range(NDC):
            nc.sync.dma_start(out=w1t[i][:], in_=W1T[i * 128:(i + 1) * 128, :])
        w2t = [ffnwp.tile([128, D], F32R, name=f"w2t{i}") for i in range(NFF)]
        for i in range(NFF):
            nc.sync.dma_start(out=w2t[i][:], in_=W2T[i * 128:(i + 1) * 128, :])


        # ---------- Fourier branch (bf16) ----------
        with tc.tile_pool(name="fr", bufs=1) as fr, \
             tc.tile_pool(name="frp", bufs=1, space="PSUM") as frp, \
             tc.tile_pool(name="psy", bufs=2, space="PSUM") as psyp, \
             tc.tile_pool(name="wpkp", bufs=2) as wpkp:
            qt = [[None] * NDC for _ in range(BLOC)]
            for b in range(BLOC):
                xbf = fr.tile([128, NLC * D], BF16, name=f"xbf{b}", tag="xbf")
                nc.sync.dma_start(out=xbf[:], in_=XBF[b])
                # DFT: xselT[d, m-ext] = sum_l x[l, d] * Fcat[l, m-ext]
                xselT = fr.tile([128, NDC * 128], BF16, name=f"xselT{b}", tag="xselT")
                for dc in range(NDC):
                    ps = frp.tile([128, 128], F32, name="psA", tag="psA")
                    for lc in range(NLC):
                        nc.tensor.matmul(
                            ps[:],
                            xbf[:, lc * D + dc * 128: lc * D + (dc + 1) * 128],
                            fct[:, lc * 128:(lc + 1) * 128],
                            start=(lc == 0), stop=(lc == NLC - 1))
                    nc.scalar.copy(xselT[:, dc * 128:(dc + 1) * 128], ps[:])
                # q-projection in mode space: QT[dout, m-ext]
                for do in range(NDC):
                    qt[b][do] = fr.tile([128, 128], BF16, name=f"qt{b}_{do}",
                                        tag=f"qt{b}_{do}")
                    ps = frp.tile([128, 128], F32, name="psQ", tag="psA")
                    for dc in range(NDC):
                        nc.tensor.matmul(
                            ps[:], wqt[dc][:, do * 128:(do + 1) * 128],
                            xselT[:, dc * 128:(dc + 1) * 128],
                            start=(dc == 0), stop=(dc == NDC - 1))
                    if need_bq:
                        nc.vector.tensor_tensor(
                            ps[:, j0:j0 + 1], ps[:, j0:j0 + 1],
                            bq4[:, do:do + 1], OP.add)
                    nc.scalar.copy(qt[b][do][:], ps[:])

            # mode mix: per head, per mode, complex ExE channel mix.
            # RH_h rows: 0:64 = Qre e-rows, 64:128 = Qim e-rows; col = 2m + b
            rh = [fr.tile([128, 128], BF16, name=f"rh{h}", tag=f"rh{h}")
                  for h in range(H)]
            for h in range(H):
                src_do, r0 = h // 2, (h % 2) * 64
                for b in range(BLOC):
                    rhv = rh[h].rearrange("p (m t) -> p m t", t=2)
                    nc.scalar.copy(rhv[0:64, :, b], qt[b][src_do][r0:r0 + 64, 0:64])
                    nc.scalar.copy(rhv[64:128, :, b], qt[b][src_do][r0:r0 + 64, 64:128])
            otre = [[fr.tile([128, M], BF16, name=f"otre{b}_{dc}", tag=f"otre{b}{dc}")
                     for dc in range(NDC)] for b in range(BLOC)]
            otim = [[fr.tile([128, M], BF16, name=f"otim{b}_{dc}", tag=f"otim{b}{dc}")
                     for dc in range(NDC)] for b in range(BLOC)]
            for h in range(H):
                psm = frp.tile([128, 128], F32, name="psM", tag="psM")
                for q in range(4):
                    wpk_q = wpkp.tile([128, 16 * 128], BF16, name=f"wpk{h}_{q}",
                                      tag="wpk")
                    nc.sync.dma_start(out=wpk_q[:],
                                      in_=WPK[h][:, q * 2048:(q + 1) * 2048])
                    for mq in range(16):
                        m = q * 16 + mq
                        nc.tensor.matmul(
                            psm[:, 2 * m:2 * m + 2],
                            wpk_q[:, mq * 128:(mq + 1) * 128],
                            rh[h][:, 2 * m:2 * m + 2],
                            start=True, stop=True)
                psv = psm.rearrange("p (m t) -> p m t", t=2)
                dc, r0 = h // 2, (h % 2) * 64
                for b in range(BLOC):
                    nc.scalar.copy(otre[b][dc][r0:r0 + 64, :], psv[0:64, :, b])
                    nc.scalar.copy(otim[b][dc][r0:r0 + 64, :], psv[64:128, :, b])

            # Wo projection in mode space, then transpose into pcat_b
            pcat = [fr.tile([128, D], BF16, name=f"pcat{b}", tag=f"pcat{b}")
                    for b in range(BLOC)]
            for b in range(BLOC):
                for ro, ot in ((0, otre[b]), (64, otim[b])):
                    for do in range(NDC):
                        ps = frp.tile([128, M], F32, name="psP", tag="psP")
                        for dc in range(NDC):
                            nc.tensor.matmul(
                                ps[:], wot[dc][:, do * 128:(do + 1) * 128],
                                ot[dc][:], start=(dc == 0), stop=(dc == NDC - 1))
                        pp = fr.tile([128, M], BF16, name=f"pp{ro}_{do}", tag="pp")
                        nc.scalar.copy(pp[:], ps[:])
                        pst = frp.tile([M, 128], BF16, name="psT", tag="psT")
                        nc.tensor.transpose(pst[:], pp[:], eye[:])
                        nc.scalar.copy(pcat[b][ro:ro + 64, do * 128:(do + 1) * 128],
                                       pst[:])

            # iDFT + u = x + yW + bo   (feature-major, fp32, in place over xT)
            for b in range(BLOC):
                for dc in range(NDC):
                    for t4 in range(NTC):
                        psy = psyp.tile([128, 512], F32, name="psY", tag="psY")
                        nc.tensor.matmul(
                            psy[:], pcat[b][:, dc * 128:(dc + 1) * 128],
                            c2s2[:, t4 * 512:(t4 + 1) * 512],
                            start=True, stop=True)
                        sl = mt[b][dc][:, t4 * 512:(t4 + 1) * 512]
                        nc.vector.scalar_tensor_tensor(
                            sl, psy[:], bo4[:, dc:dc + 1], sl, OP.add, OP.add)
                for dc in range(NDC):
                    decomp(decp, b, dc, 0, 1)

        # ---------- FFN (f32r) ----------
        with tc.tile_pool(name="ffa", bufs=1) as ffa, \
             tc.tile_pool(name="gqp", bufs=2) as gqp, \
             tc.tile_pool(name="pshp", bufs=2, space="PSUM") as pshp, \
             tc.tile_pool(name="psfp", bufs=1, space="PSUM") as psfp:
            for b in range(BLOC):
                for t4 in range(NTC):
                    r1c = [ffa.tile([128, 512], F32R, name=f"r1c{dc}", tag=f"r1c{dc}")
                           for dc in range(NDC)]
                    for dc in range(NDC):
                        nc.vector.tensor_copy(
                            r1c[dc][:], mt[b][dc][:, t4 * 512:(t4 + 1) * 512])
                    psf = [psfp.tile([128, 512], F32, name=f"psF{do}", tag=f"psF{do}")
                           for do in range(NDC)]
                    for ff in range(NFF):
                        psh = pshp.tile([128, 512], F32, name="psH", tag="psH")
                        for dc in range(NDC):
                            nc.tensor.matmul(
                                psh[:], w1t[dc][:, ff * 128:(ff + 1) * 128],
                                r1c[dc][:], start=(dc == 0), stop=(dc == NDC - 1))
                        gq = gqp.tile([128, 512], F32R, name="gq", tag="gq")
                        nc.scalar.activation(gq[:], psh[:], AF.Gelu)
                        for do in range(NDC):
                            nc.tensor.matmul(
                                psf[do][:], w2t[ff][:, do * 128:(do + 1) * 128],
                                gq[:], start=(ff == 0), stop=(ff == NFF - 1))
                    for do in range(NDC):
                        sl = mt[b][do][:, t4 * 512:(t4 + 1) * 512]
                        nc.vector.tensor_tensor(sl, psf[do][:], sl, OP.add)

        for b in range(BLOC):
            for dc in range(NDC):
                decomp(decp, b, dc, 2, 3)
                nc.sync.dma_start(out=OUT_T[b, dc * 128:(dc + 1) * 128, :],
                                  in_=mt[b][dc][:])

        ffnw.__exit__(None, None, None)
        decp_cm.__exit__(None, None, None)
        main.__exit__(None, None, None)
        cst.__exit__(None, None, None)

    if fix:
        _fix_sync_waits(nc)
    return nc


def _host_prep(inputs):
    import ml_dtypes
    bf16 = ml_dtypes.bfloat16
    x = np.asarray(inputs["x"], np.float32)
    modes = np.asarray(inputs["mode_index"]).astype(np.int64)
    l = np.arange(L, dtype=np.float64)
    ang = 2.0 * np.pi * np.outer(l, modes.astype(np.float64)) / L
    FC = np.concatenate([np.cos(ang), -np.sin(ang)], axis=1)          # [L, 128]
    m_out = np.arange(M, dtype=np.float64)
    w = np.where(m_out == 0, 1.0, 2.0) / L
    ang2 = 2.0 * np.pi * np.outer(m_out, l) / L
    C2 = np.concatenate([w[:, None] * np.cos(ang2),
                         w[:, None] * -np.sin(ang2)], axis=0)         # [128, L]

    FCT = FC.reshape(NLC, 128, 128).transpose(1, 0, 2).reshape(128, NLC * 128)

    wr = np.asarray(inputs["four_wr"], np.float64)   # [H, E, O, M]
    wi = np.asarray(inputs["four_wi"], np.float64)
    wpk = np.zeros((H, M, 128, 128), np.float64)
    wpk[:, :, 0:64, 0:64] = wr.transpose(0, 3, 1, 2)
    wpk[:, :, 0:64, 64:128] = wi.transpose(0, 3, 1, 2)
    wpk[:, :, 64:128, 0:64] = -wi.transpose(0, 3, 1, 2)
    wpk[:, :, 64:128, 64:128] = wr.transpose(0, 3, 1, 2)
    WPKh = wpk.transpose(0, 2, 1, 3).reshape(H, 128, M * 128)

    dec1_w = np.asarray(inputs["dec1_w"], np.float64)
    dec1_b = np.asarray(inputs["dec1_b"], np.float64)
    dec2_w = np.asarray(inputs["dec2_w"], np.float64)
    dec2_b = np.asarray(inputs["dec2_b"], np.float64)
    decs = np.zeros((128, 4), np.float32)
    decs[:, 0] = dec1_w[0] - dec1_w[1]
    decs[:, 1] = dec1_b[0] - dec1_b[1]
    decs[:, 2] = dec2_w[0] - dec2_w[1]
    decs[:, 3] = dec2_b[0] - dec2_b[1]

    bo = np.asarray(inputs["bo"], np.float32)
    bq = np.asarray(inputs["bq"], np.float32)
    BO4 = np.ascontiguousarray(bo.reshape(NDC, 128).T).astype(np.float32)
    zero_pos = np.nonzero(modes == 0)[0]
    need_bq = bool(len(zero_pos)) and bool(np.any(bq != 0))
    j0 = int(zero_pos[0]) if need_bq else 0
    BQ4 = np.ascontiguousarray((L * bq).reshape(NDC, 128).T).astype(np.float32)

    ech13 = np.tile((6.0 - np.arange(7.0))[None, :], (128, 1)).astype(np.float32)
    etl13 = np.tile((np.arange(6.0) + 1.0)[None, :], (128, 1)).astype(np.float32)
    ech25 = np.tile((12.0 - np.arange(13.0))[None, :], (128, 1)).astype(np.float32)
    etl25 = np.tile((np.arange(12.0) + 1.0)[None, :], (128, 1)).astype(np.float32)

    shared = {
        "FCT": FCT.astype(bf16),
        "C2S2": C2.astype(bf16),
        "WQT": np.ascontiguousarray(np.asarray(inputs["Wq"], np.float32).T).astype(bf16),
        "WOT": np.ascontiguousarray(np.asarray(inputs["Wo"], np.float32).T).astype(bf16),
        "WPK": WPKh.astype(bf16),
        "W1T": np.ascontiguousarray(np.asarray(inputs["conv1_w"], np.float32).T),
        "W2T": np.ascontiguousarray(np.asarray(inputs["conv2_w"], np.float32).T),
        "EYE": np.eye(128, dtype=np.float32).astype(bf16),
        "BO4": BO4, "BQ4": BQ4,
        "ECH13": ech13, "ETL13": etl13, "ECH25": ech25, "ETL25": etl25,
        "DECS": decs,
    }
    in_maps = []
    for c in range(NC_):
        xl = x[c * BLOC:(c + 1) * BLOC]                       # [2, L, D]
        XTc = np.ascontiguousarray(xl.transpose(0, 2, 1))     # [2, D, L]
        xbf = xl.astype(bf16)                                 # [2, L, D]
        XBFc = np.ascontiguousarray(
            xbf.reshape(BLOC, NLC, 128, D).transpose(0, 2, 1, 3)
        ).reshape(BLOC, 128, NLC * D)
        im = dict(shared)
        im["XT"] = XTc
        im["XBF"] = XBFc
        in_maps.append(im)
    return in_maps, need_bq, j0


def kernel(**inputs):
    from concourse.bass_utils import run_bass_kernel_spmd

    in_maps, need_bq, j0 = _host_prep(inputs)
    key = (need_bq, j0)
    if key not in _prog_cache:
        _prog_cache[key] = _build_program(need_bq, j0)
    nc = _prog_cache[key]
    res = run_bass_kernel_spmd(nc, in_maps, core_ids=list(range(NC_)))
    outs = []
    for c in range(NC_):
        ot = np.asarray(res.results[c]["OUT_T"])              # [2, D, L]
        outs.append(np.ascontiguousarray(ot.transpose(0, 2, 1)))
    return np.concatenate(outs, axis=0).astype(np.float32)



# revision 5
# speedup vs baseline: 1.1926x; 1.1926x over previous
"""FEDformer encoder layer on 8 TRN2 NeuronCores — batch-data-parallel Bass kernel.

Strategy (self-contained; shapes hardcoded):
  B=16,L=2048,D=512,H=8,E=64,M=64,DFF=2048; 8 cores x 2 batches each; no collectives.

  Math restructuring (validated against the jax reference):
   - rfft+mode-gather == x @ Fcat where Fcat[l, 0:64]=cos(2*pi*k_j*l/L),
     Fcat[l, 64:128]=-sin(...), k_j = mode_index.
   - Wq and Wo commute with the DFT -> applied in mode space (16x cheaper).
     k/v projections are dead code in the reference.
   - irfft of a spectrum with only bins 0..63 populated == P @ C2S2.
   - Fourier branch contributes ~1e-5 absolute to an O(1) output -> bf16 there.
   - series-decomp: K=2 softmax == sigmoid of weight/bias deltas; moving
     sums via a scan-free bf16 shift-add chain (s2,s4,s8,s12,s13,s25) over a
     replicate-padded window buffer; mean = ma25 + g*(ma13-ma25); r = u - mean.
   - FFN in bf16 (weights + activations), gelu via ACT LUT.

  Pipelining: chunked along L into 4 chunks of 512 tokens. Per batch:
  iDFT(c) -> decomp1(c) -> FFN(c) -> decomp2(c) -> DMA-out(c), emitted so the
  tensor engine streams FFN matmuls while DVE/GpSimd run the decompositions
  of neighboring chunks. decomp2 writes a separate bf16 tile (not in-place)
  so chunk c's output never clobbers the halo tokens chunk c+1 still reads.

  Layout: device works feature-major ([D, L]); the host transposes x in and the
  output back during shard/unshard.
"""

import numpy as np

B, L, D, H, M, DFF = 16, 2048, 512, 8, 64, 2048
E = D // H
NC_ = 8
BLOC = B // NC_          # batches per core
MEXT = 2 * M             # re|im rows
NDC = D // 128           # 4 feature tiles
NFF = DFF // 128         # 16 dff tiles
NLC = L // 128           # 16 token chunks of 128
NTC = L // 512           # 4 token chunks of 512
CH = 512                 # decomp/FFN chunk length
PAD = 12                 # replicate halo for the 25-window
UPW = CH + 2 * PAD       # 536

_prog_cache = {}
_fixn = [0]


def _fix_sync_waits(nc, max_waits=1, max_updates=4):
    """Split >max sem-waits/updates per instruction onto adjacent nops."""
    import concourse.mybir as mybir

    for f in nc.m.functions:
        for bb in f.blocks:
            insts = bb.instructions
            i = 0
            while i < len(insts):
                ins = insts[i]
                si = ins.sync_info
                if si is not None and si.on_wait and len(si.on_wait) > max_waits:
                    waits = list(si.on_wait)
                    si.on_wait = waits[-max_waits:]
                    rest = waits[:-max_waits]
                    chunks = [rest[j:j + max_waits]
                              for j in range(0, len(rest), max_waits)]
                    for c in reversed(chunks):
                        _fixn[0] += 1
                        nop = mybir.InstNoOp(name=f"I-fixw-{_fixn[0]}", ins=[], outs=[])
                        nop.engine = ins.engine
                        nop.sync_info = mybir.SyncInfo(on_wait=c, on_update=[])
                        insts.insert(i, nop)
                        i += 1
                if si is not None and si.on_update and len(si.on_update) > max_updates:
                    ups = list(si.on_update)
                    si.on_update = ups[:max_updates]
                    rest = ups[max_updates:]
                    chunks = [rest[j:j + max_updates]
                              for j in range(0, len(rest), max_updates)]
                    for c in chunks:
                        _fixn[0] += 1
                        nop = mybir.InstNoOp(name=f"I-fixu-{_fixn[0]}", ins=[], outs=[])
                        nop.engine = ins.engine
                        nop.sync_info = mybir.SyncInfo(on_wait=[], on_update=c)
                        insts.insert(i + 1, nop)
                        i += 1
                i += 1


def _build_program(need_bq, j0, fix=True):
    import concourse.bass as bass
    import concourse.mybir as mybir
    from concourse.tile import TileContext

    F32 = mybir.dt.float32
    BF16 = mybir.dt.bfloat16
    AF = mybir.ActivationFunctionType
    OP = mybir.AluOpType

    nc = bass.Bass()

    # ---- DRAM I/O ----
    XT = nc.dram_tensor("XT", [BLOC, D, L], F32, kind="ExternalInput")
    XBF = nc.dram_tensor("XBF", [BLOC, 128, NLC * D], BF16, kind="ExternalInput")
    FCT = nc.dram_tensor("FCT", [128, NLC * 128], BF16, kind="ExternalInput")
    C2S2 = nc.dram_tensor("C2S2", [128, L], BF16, kind="ExternalInput")
    WQT = nc.dram_tensor("WQT", [D, D], BF16, kind="ExternalInput")
    WOT = nc.dram_tensor("WOT", [D, D], BF16, kind="ExternalInput")
    WPK = nc.dram_tensor("WPK", [H, 128, M * 128], BF16, kind="ExternalInput")
    W1T = nc.dram_tensor("W1T", [D, DFF], BF16, kind="ExternalInput")
    W2T = nc.dram_tensor("W2T", [DFF, D], BF16, kind="ExternalInput")
    EYE = nc.dram_tensor("EYE", [128, 128], BF16, kind="ExternalInput")
    BO4 = nc.dram_tensor("BO4", [128, NDC], F32, kind="ExternalInput")
    BQ4 = nc.dram_tensor("BQ4", [128, NDC], F32, kind="ExternalInput")
    DECS = nc.dram_tensor("DECS", [128, 4], F32, kind="ExternalInput")
    OUT_T = nc.dram_tensor("OUT_T", [BLOC, D, L], BF16, kind="ExternalOutput")

    with TileContext(nc) as tc:
        # ---------- persistent pools (LIFO close at the end) ----------
        cst = tc.tile_pool(name="cst", bufs=1)
        cstp = cst.__enter__()

        # small constants first: DFT can start as soon as fct + xbf land
        fct = cstp.tile([128, NLC * 128], BF16, name="fct")
        nc.sync.dma_start(out=fct[:], in_=FCT[:])
        wqt = [cstp.tile([128, D], BF16, name=f"wqt{i}") for i in range(NDC)]
        for i in range(NDC):
            nc.sync.dma_start(out=wqt[i][:], in_=WQT[i * 128:(i + 1) * 128, :])
        c2s2 = cstp.tile([128, L], BF16, name="c2s2")
        nc.sync.dma_start(out=c2s2[:], in_=C2S2[:])
        wot = [cstp.tile([128, D], BF16, name=f"wot{i}") for i in range(NDC)]
        for i in range(NDC):
            nc.sync.dma_start(out=wot[i][:], in_=WOT[i * 128:(i + 1) * 128, :])
        eye = cstp.tile([128, 128], BF16, name="eye")
        nc.sync.dma_start(out=eye[:], in_=EYE[:])
        bo4 = cstp.tile([128, NDC], F32, name="bo4")
        nc.sync.dma_start(out=bo4[:], in_=BO4[:])
        decs = cstp.tile([128, 4], F32, name="decs")
        nc.sync.dma_start(out=decs[:], in_=DECS[:])
        bq4 = None
        if need_bq:
            bq4 = cstp.tile([128, NDC], F32, name="bq4")
            nc.sync.dma_start(out=bq4[:], in_=BQ4[:])
        ones12 = cstp.tile([128, PAD], BF16, name="ones12")
        nc.vector.memset(ones12[:], 1.0)

        main = tc.tile_pool(name="main", bufs=1)
        mainp = main.__enter__()
        # main fp32 buffer: x -> u -> z=r1+ffn (in place)
        mt = [[mainp.tile([128, L], F32, name=f"m_{b}_{dc}") for dc in range(NDC)]
              for b in range(BLOC)]
        # decomp1 output (FFN input), bf16
        r1 = [[mainp.tile([128, L], BF16, name=f"r1_{b}_{dc}") for dc in range(NDC)]
              for b in range(BLOC)]

        decp_cm = tc.tile_pool(name="decp", bufs=2)
        decp = decp_cm.__enter__()

        # ---------- scan-free chunked series decomposition ----------
        def gate_chunk(b, dc, c, dw_col, db_col):
            src = mt[b][dc]
            c0 = c * CH
            g = decp.tile([128, CH], BF16, name=f"g{dc}", tag=f"g{dc}")
            nc.scalar.activation(g[:], src[:, c0:c0 + CH], AF.Sigmoid,
                                 scale=decs[:, dw_col:dw_col + 1],
                                 bias=decs[:, db_col:db_col + 1])
            return g

        def decomp_chunk(b, dc, c, g, dst, par):
            """src = mt[b][dc] fp32; writes residual chunk c to dst slice."""
            src = mt[b][dc]
            c0 = c * CH
            up = decp.tile([128, UPW], BF16, name="up", tag="up")
            if c == 0:
                nc.vector.tensor_copy(up[:, PAD:UPW], src[:, 0:CH + PAD])
                nc.vector.tensor_scalar_mul(up[:, 0:PAD], ones12[:], src[:, 0:1])
            elif c == NTC - 1:
                nc.vector.tensor_copy(up[:, 0:UPW - PAD], src[:, c0 - PAD:L])
                nc.vector.tensor_scalar_mul(up[:, UPW - PAD:UPW], ones12[:],
                                            src[:, L - 1:L])
            else:
                nc.vector.tensor_copy(up[:, 0:UPW], src[:, c0 - PAD:c0 + CH + PAD])
            # windowed sums via shift-adds (all bf16)
            s2 = decp.tile([128, 535], BF16, name="s2", tag="s2")
            nc.gpsimd.tensor_tensor(s2[:], up[:, 0:535], up[:, 1:536], OP.add)
            s4 = decp.tile([128, 533], BF16, name="s4", tag="s4")
            e4 = nc.gpsimd if par else nc.vector
            e4.tensor_tensor(s4[:], s2[:, 0:533], s2[:, 2:535], OP.add)
            s8 = decp.tile([128, 529], BF16, name="s8", tag="s8")
            nc.vector.tensor_tensor(s8[:], s4[:, 0:529], s4[:, 4:533], OP.add)
            s12 = decp.tile([128, 525], BF16, name="s12", tag="s12")
            nc.vector.tensor_tensor(s12[:], s8[:, 0:525], s4[:, 8:533], OP.add)
            s13 = decp.tile([128, 524], BF16, name="s13", tag="s13")
            nc.vector.tensor_tensor(s13[:], s12[:, 0:524], up[:, 12:536], OP.add)
            s25 = decp.tile([128, CH], BF16, name="s25", tag="s25")
            nc.vector.tensor_tensor(s25[:], s12[:, 0:CH], s13[:, 12:12 + CH], OP.add)
            ma = decp.tile([128, CH], BF16, name="ma", tag="ma")
            nc.vector.tensor_scalar_mul(ma[:], s25[:], 1.0 / 25.0)
            dl = decp.tile([128, CH], BF16, name="dl", tag="dl")
            nc.vector.scalar_tensor_tensor(dl[:], s13[:, 6:6 + CH], 1.0 / 13.0,
                                           ma[:], OP.mult, OP.subtract)
            e = decp.tile([128, CH], BF16, name="e", tag="e")
            nc.gpsimd.tensor_tensor(e[:], g[:], dl[:], OP.mult)
            t1 = decp.tile([128, CH], BF16, name="t1", tag="dl")
            nc.vector.tensor_tensor(t1[:], ma[:], e[:], OP.add)
            er = nc.vector if par else nc.gpsimd
            er.tensor_tensor(dst, src[:, c0:c0 + CH], t1[:], OP.subtract)

        # ---------- Fourier front: DFT + q-projection (both batches) ----------
        fr_cm = tc.tile_pool(name="fr", bufs=1)
        fr = fr_cm.__enter__()
        frp_cm = tc.tile_pool(name="frp", bufs=2, space="PSUM")
        frp = frp_cm.__enter__()
        xbf_cm = tc.tile_pool(name="xbfp", bufs=1)
        xbfp = xbf_cm.__enter__()

        qt = [[None] * NDC for _ in range(BLOC)]
        for b in range(BLOC):
            xbf = xbfp.tile([128, NLC * D], BF16, name=f"xbf{b}", tag="xbf")
            for gch in range(4):
                nc.sync.dma_start(out=xbf[:, gch * 4 * D:(gch + 1) * 4 * D],
                                  in_=XBF[b][:, gch * 4 * D:(gch + 1) * 4 * D])
            psA = frp.tile([128, 128], F32, name="psA", tag="fps")
            xselT = fr.tile([128, NDC * 128], BF16, name=f"xselT{b}", tag=f"xselT{b}")
            for dc in range(NDC):
                for lc in range(NLC):
                    nc.tensor.matmul(
                        psA[:],
                        xbf[:, lc * D + dc * 128: lc * D + (dc + 1) * 128],
                        fct[:, lc * 128:(lc + 1) * 128],
                        start=(lc == 0), stop=(lc == NLC - 1))
                nc.vector.tensor_copy(xselT[:, dc * 128:(dc + 1) * 128], psA[:])
                psA = frp.tile([128, 128], F32, name="psA", tag="fps")
            # q-projection in mode space
            for do in range(NDC):
                qt[b][do] = fr.tile([128, 128], BF16, name=f"qt{b}_{do}",
                                    tag=f"qt{b}_{do}")
                ps = frp.tile([128, 128], F32, name="psQ", tag="fps")
                for dc in range(NDC):
                    nc.tensor.matmul(
                        ps[:], wqt[dc][:, do * 128:(do + 1) * 128],
                        xselT[:, dc * 128:(dc + 1) * 128],
                        start=(dc == 0), stop=(dc == NDC - 1))
                if need_bq:
                    nc.vector.tensor_tensor(
                        ps[:, j0:j0 + 1], ps[:, j0:j0 + 1],
                        bq4[:, do:do + 1], OP.add)
                nc.vector.tensor_copy(qt[b][do][:], ps[:])
        xbf_cm.__exit__(None, None, None)

        # ---------- mode mix (both batches packed into columns) ----------
        wpk_cm = tc.tile_pool(name="wpkp", bufs=2)
        wpkp = wpk_cm.__enter__()
        rh = [fr.tile([128, 128], BF16, name=f"rh{h}", tag=f"rh{h}")
              for h in range(H)]
        for h in range(H):
            src_do, r0 = h // 2, (h % 2) * 64
            for b in range(BLOC):
                rhv = rh[h].rearrange("p (m t) -> p m t", t=2)
                nc.vector.tensor_copy(rhv[0:64, :, b], qt[b][src_do][r0:r0 + 64, 0:64])
                nc.vector.tensor_copy(rhv[64:128, :, b],
                                      qt[b][src_do][r0:r0 + 64, 64:128])
        otre = [[fr.tile([128, M], BF16, name=f"otre{b}_{dc}", tag=f"otre{b}{dc}")
                 for dc in range(NDC)] for b in range(BLOC)]
        otim = [[fr.tile([128, M], BF16, name=f"otim{b}_{dc}", tag=f"otim{b}{dc}")
                 for dc in range(NDC)] for b in range(BLOC)]
        for h in range(H):
            psm = frp.tile([128, 128], F32, name="psM", tag="fps")
            for q in range(8):
                wpk_q = wpkp.tile([128, 8 * 128], BF16, name=f"wpk{h}_{q}",
                                  tag="wpk")
                nc.sync.dma_start(out=wpk_q[:],
                                  in_=WPK[h][:, q * 1024:(q + 1) * 1024])
                for mq in range(8):
                    m = q * 8 + mq
                    nc.tensor.matmul(
                        psm[:, 2 * m:2 * m + 2],
                        wpk_q[:, mq * 128:(mq + 1) * 128],
                        rh[h][:, 2 * m:2 * m + 2],
                        start=True, stop=True)
            psv = psm.rearrange("p (m t) -> p m t", t=2)
            dc, r0 = h // 2, (h % 2) * 64
            for b in range(BLOC):
                nc.vector.tensor_copy(otre[b][dc][r0:r0 + 64, :], psv[0:64, :, b])
                nc.vector.tensor_copy(otim[b][dc][r0:r0 + 64, :], psv[64:128, :, b])
        wpk_cm.__exit__(None, None, None)

        # FFN weights arrive during the mode-mix / decomp1 window
        ffnw = tc.tile_pool(name="ffnw", bufs=1)
        ffnwp = ffnw.__enter__()
        w1t = [ffnwp.tile([128, DFF], BF16, name=f"w1t{i}") for i in range(NDC)]
        for i in range(NDC):
            nc.sync.dma_start(out=w1t[i][:], in_=W1T[i * 128:(i + 1) * 128, :])
        w2t = [ffnwp.tile([128, D], BF16, name=f"w2t{i}") for i in range(NFF)]
        for i in range(NFF):
            nc.sync.dma_start(out=w2t[i][:], in_=W2T[i * 128:(i + 1) * 128, :])
        # x^T arrives late (only needed at the iDFT residual add)
        for b in range(BLOC):
            for dc in range(NDC):
                nc.sync.dma_start(out=mt[b][dc][:],
                                  in_=XT[b, dc * 128:(dc + 1) * 128, :])

        # ---------- per-batch pipeline: Wo -> iDFT -> d1 -> FFN -> d2 ----------
        psy_cm = tc.tile_pool(name="psy", bufs=1, space="PSUM")
        psyp = psy_cm.__enter__()
        gq_cm = tc.tile_pool(name="gqp", bufs=16)
        gqp = gq_cm.__enter__()
        psh_cm = tc.tile_pool(name="pshp", bufs=2, space="PSUM")
        pshp = psh_cm.__enter__()
        psf_cm = tc.tile_pool(name="psfp", bufs=2, space="PSUM")
        psfp = psf_cm.__enter__()

        def wo_proj(b):
            pcat = fr.tile([128, D], BF16, name=f"pcat{b}", tag=f"pcat{b}")
            for ro, ot in ((0, otre[b]), (64, otim[b])):
                for do in range(NDC):
                    ps = frp.tile([128, M], F32, name="psP", tag="fps")
                    for dc in range(NDC):
                        nc.tensor.matmul(
                            ps[:], wot[dc][:, do * 128:(do + 1) * 128],
                            ot[dc][:], start=(dc == 0), stop=(dc == NDC - 1))
                    pp = fr.tile([128, M], BF16, name=f"pp{ro}_{do}", tag="pp")
                    nc.scalar.activation(pp[:], ps[:], AF.Copy)
                    pst = frp.tile([M, 128], BF16, name="psT", tag="fps")
                    nc.tensor.transpose(pst[:], pp[:], eye[:])
                    nc.scalar.activation(
                        pcat[ro:ro + 64, do * 128:(do + 1) * 128], pst[:], AF.Copy)
            return pcat

        def idft_chunk(b, pcat, c):
            for dc in range(NDC):
                psy = psyp.tile([128, CH], F32, name="psY", tag="psY")
                nc.tensor.matmul(
                    psy[:], pcat[:, dc * 128:(dc + 1) * 128],
                    c2s2[:, c * CH:(c + 1) * CH],
                    start=True, stop=True)
                sl = mt[b][dc][:, c * CH:(c + 1) * CH]
                nc.vector.scalar_tensor_tensor(
                    sl, psy[:], bo4[:, dc:dc + 1], sl, OP.add, OP.add)

        def ffn_chunk(b, c):
            gqs = []
            for ff in range(NFF):
                psh = pshp.tile([128, CH], F32, name="psH", tag="psH")
                for dc in range(NDC):
                    nc.tensor.matmul(
                        psh[:], w1t[dc][:, ff * 128:(ff + 1) * 128],
                        r1[b][dc][:, c * CH:(c + 1) * CH],
                        start=(dc == 0), stop=(dc == NDC - 1))
                gqv = gqp.tile([128, CH], BF16, name="gq", tag="gq")
                nc.scalar.activation(gqv[:], psh[:], AF.Gelu)
                gqs.append(gqv)
            for do in range(NDC):
                psf = psfp.tile([128, CH], F32, name="psF", tag="psF")
                for ff in range(NFF):
                    nc.tensor.matmul(
                        psf[:], w2t[ff][:, do * 128:(do + 1) * 128],
                        gqs[ff][:], start=(ff == 0), stop=(ff == NFF - 1))
                sl = mt[b][do][:, c * CH:(c + 1) * CH]
                nc.vector.tensor_tensor(sl, psf[:],
                                        r1[b][do][:, c * CH:(c + 1) * CH], OP.add)

        def d1_chunk(b, c):
            gs = [gate_chunk(b, dc, c, 0, 1) for dc in range(NDC)]
            for dc in range(NDC):
                decomp_chunk(b, dc, c, gs[dc],
                             r1[b][dc][:, c * CH:(c + 1) * CH], (dc + c) % 2)

        def d2_chunk(b, c):
            gs = [gate_chunk(b, dc, c, 2, 3) for dc in range(NDC)]
            for dc in range(NDC):
                ob = decp.tile([128, CH], BF16, name="ob", tag="ob")
                decomp_chunk(b, dc, c, gs[dc], ob[:], (dc + c) % 2)
                nc.sync.dma_start(
                    out=OUT_T[b, dc * 128:(dc + 1) * 128, c * CH:(c + 1) * CH],
                    in_=ob[:])

        for b in range(BLOC):
            pcat = wo_proj(b)
            idft_chunk(b, pcat, 0)
            idft_chunk(b, pcat, 1)
            d1_chunk(b, 0)
            idft_chunk(b, pcat, 2)
            d1_chunk(b, 1)
            ffn_chunk(b, 0)
            idft_chunk(b, pcat, 3)
            d1_chunk(b, 2)
            ffn_chunk(b, 1)
            d1_chunk(b, 3)
            d2_chunk(b, 0)
            ffn_chunk(b, 2)
            d2_chunk(b, 1)
            ffn_chunk(b, 3)
            d2_chunk(b, 2)
            d2_chunk(b, 3)

        psf_cm.__exit__(None, None, None)
        psh_cm.__exit__(None, None, None)
        gq_cm.__exit__(None, None, None)
        psy_cm.__exit__(None, None, None)
        ffnw.__exit__(None, None, None)
        frp_cm.__exit__(None, None, None)
        fr_cm.__exit__(None, None, None)
        decp_cm.__exit__(None, None, None)
        main.__exit__(None, None, None)
        cst.__exit__(None, None, None)

    if fix:
        _fix_sync_waits(nc)
    return nc


def _host_prep(inputs):
    import ml_dtypes
    bf16 = ml_dtypes.bfloat16
    x = np.asarray(inputs["x"], np.float32)
    modes = np.asarray(inputs["mode_index"]).astype(np.int64)
    l = np.arange(L, dtype=np.float64)
    ang = 2.0 * np.pi * np.outer(l, modes.astype(np.float64)) / L
    FC = np.concatenate([np.cos(ang), -np.sin(ang)], axis=1)          # [L, 128]
    m_out = np.arange(M, dtype=np.float64)
    w = np.where(m_out == 0, 1.0, 2.0) / L
    ang2 = 2.0 * np.pi * np.outer(m_out, l) / L
    C2 = np.concatenate([w[:, None] * np.cos(ang2),
                         w[:, None] * -np.sin(ang2)], axis=0)         # [128, L]

    FCT = FC.reshape(NLC, 128, 128).transpose(1, 0, 2).reshape(128, NLC * 128)

    wr = np.asarray(inputs["four_wr"], np.float64)   # [H, E, O, M]
    wi = np.asarray(inputs["four_wi"], np.float64)
    wpk = np.zeros((H, M, 128, 128), np.float64)
    wpk[:, :, 0:64, 0:64] = wr.transpose(0, 3, 1, 2)
    wpk[:, :, 0:64, 64:128] = wi.transpose(0, 3, 1, 2)
    wpk[:, :, 64:128, 0:64] = -wi.transpose(0, 3, 1, 2)
    wpk[:, :, 64:128, 64:128] = wr.transpose(0, 3, 1, 2)
    WPKh = wpk.transpose(0, 2, 1, 3).reshape(H, 128, M * 128)

    dec1_w = np.asarray(inputs["dec1_w"], np.float64)
    dec1_b = np.asarray(inputs["dec1_b"], np.float64)
    dec2_w = np.asarray(inputs["dec2_w"], np.float64)
    dec2_b = np.asarray(inputs["dec2_b"], np.float64)
    decs = np.zeros((128, 4), np.float32)
    decs[:, 0] = dec1_w[0] - dec1_w[1]
    decs[:, 1] = dec1_b[0] - dec1_b[1]
    decs[:, 2] = dec2_w[0] - dec2_w[1]
    decs[:, 3] = dec2_b[0] - dec2_b[1]

    bo = np.asarray(inputs["bo"], np.float32)
    bq = np.asarray(inputs["bq"], np.float32)
    BO4 = np.ascontiguousarray(bo.reshape(NDC, 128).T).astype(np.float32)
    zero_pos = np.nonzero(modes == 0)[0]
    need_bq = bool(len(zero_pos)) and bool(np.any(bq != 0))
    j0 = int(zero_pos[0]) if need_bq else 0
    BQ4 = np.ascontiguousarray((L * bq).reshape(NDC, 128).T).astype(np.float32)

    shared = {
        "FCT": FCT.astype(bf16),
        "C2S2": C2.astype(bf16),
        "WQT": np.ascontiguousarray(np.asarray(inputs["Wq"], np.float32).T).astype(bf16),
        "WOT": np.ascontiguousarray(np.asarray(inputs["Wo"], np.float32).T).astype(bf16),
        "WPK": WPKh.astype(bf16),
        "W1T": np.ascontiguousarray(np.asarray(inputs["conv1_w"], np.float32).T).astype(bf16),
        "W2T": np.ascontiguousarray(np.asarray(inputs["conv2_w"], np.float32).T).astype(bf16),
        "EYE": np.eye(128, dtype=np.float32).astype(bf16),
        "BO4": BO4, "BQ4": BQ4,
        "DECS": decs,
    }
    in_maps = []
    for c in range(NC_):
        xl = x[c * BLOC:(c + 1) * BLOC]                       # [2, L, D]
        XTc = np.ascontiguousarray(xl.transpose(0, 2, 1))     # [2, D, L]
        xbf = xl.astype(bf16)                                 # [2, L, D]
        XBFc = np.ascontiguousarray(
            xbf.reshape(BLOC, NLC, 128, D).transpose(0, 2, 1, 3)
        ).reshape(BLOC, 128, NLC * D)
        im = dict(shared)
        im["XT"] = XTc
        im["XBF"] = XBFc
        in_maps.append(im)
    return in_maps, need_bq, j0


def kernel(**inputs):
    from concourse.bass_utils import run_bass_kernel_spmd

    in_maps, need_bq, j0 = _host_prep(inputs)
    key = (need_bq, j0)
    if key not in _prog_cache:
        _prog_cache[key] = _build_program(need_bq, j0)
    nc = _prog_cache[key]
    res = run_bass_kernel_spmd(nc, in_maps, core_ids=list(range(NC_)))
    outs = []
    for c in range(NC_):
        ot = np.asarray(res.results[c]["OUT_T"]).astype(np.float32)  # [2, D, L]
        outs.append(np.ascontiguousarray(ot.transpose(0, 2, 1)))
    return np.concatenate(outs, axis=0).astype(np.float32)


# revision 8
# speedup vs baseline: 1.4444x; 1.2112x over previous
"""FEDformer encoder layer on 8 TRN2 NeuronCores — batch-data-parallel Bass kernel.

Strategy (self-contained; shapes hardcoded):
  B=16,L=2048,D=512,H=8,E=64,M=64,DFF=2048; 8 cores x 2 batches each; no collectives.

  Math restructuring (validated against the jax reference):
   - rfft+mode-gather == x @ Fcat where Fcat[l, 0:64]=cos(2*pi*k_j*l/L),
     Fcat[l, 64:128]=-sin(...), k_j = mode_index.
   - Wq and Wo commute with the DFT -> applied in mode space (16x cheaper).
     k/v projections are dead code in the reference.
   - irfft of a spectrum with only bins 0..63 populated == P @ C2S2.
   - Fourier branch contributes ~1e-5 absolute to an O(1) output -> bf16 there.
   - series-decomp: K=2 softmax == sigmoid of weight/bias deltas; moving
     sums via a scan-free bf16 shift-add chain (s2,s4,s8,s12,s13,s25) over a
     replicate-padded window buffer; mean = ma25 + g*(ma13-ma25); r = u - mean.
   - FFN in bf16 (weights + activations), gelu via ACT LUT.

  Pipelining: chunked along L into 4 chunks of 512 tokens. Per batch:
  iDFT(c) -> decomp1(c) -> FFN(c) -> decomp2(c) -> DMA-out(c), emitted so the
  tensor engine streams FFN matmuls while DVE/GpSimd run the decompositions
  of neighboring chunks. decomp2 writes a separate bf16 tile (not in-place)
  so chunk c's output never clobbers the halo tokens chunk c+1 still reads.

  Layout: device works feature-major ([D, L]); the host transposes x in and the
  output back during shard/unshard.
"""

import numpy as np

B, L, D, H, M, DFF = 16, 2048, 512, 8, 64, 2048
E = D // H
NC_ = 8
BLOC = B // NC_          # batches per core
MEXT = 2 * M             # re|im rows
NDC = D // 128           # 4 feature tiles
NFF = DFF // 128         # 16 dff tiles
NLC = L // 128           # 16 token chunks of 128
NTC = L // 512           # 4 token chunks of 512
CH = 512                 # decomp/FFN chunk length
PAD = 12                 # replicate halo for the 25-window
UPW = CH + 2 * PAD       # 536

_prog_cache = {}
_fixn = [0]


def _fix_sync_waits(nc, max_waits=1, max_updates=4):
    """Split >max sem-waits/updates per instruction onto adjacent nops."""
    import concourse.mybir as mybir

    for f in nc.m.functions:
        for bb in f.blocks:
            insts = bb.instructions
            i = 0
            while i < len(insts):
                ins = insts[i]
                si = ins.sync_info
                if si is not None and si.on_wait and len(si.on_wait) > max_waits:
                    waits = list(si.on_wait)
                    si.on_wait = waits[-max_waits:]
                    rest = waits[:-max_waits]
                    chunks = [rest[j:j + max_waits]
                              for j in range(0, len(rest), max_waits)]
                    for c in reversed(chunks):
                        _fixn[0] += 1
                        nop = mybir.InstNoOp(name=f"I-fixw-{_fixn[0]}", ins=[], outs=[])
                        nop.engine = ins.engine
                        nop.sync_info = mybir.SyncInfo(on_wait=c, on_update=[])
                        insts.insert(i, nop)
                        i += 1
                if si is not None and si.on_update and len(si.on_update) > max_updates:
                    ups = list(si.on_update)
                    si.on_update = ups[:max_updates]
                    rest = ups[max_updates:]
                    chunks = [rest[j:j + max_updates]
                              for j in range(0, len(rest), max_updates)]
                    for c in chunks:
                        _fixn[0] += 1
                        nop = mybir.InstNoOp(name=f"I-fixu-{_fixn[0]}", ins=[], outs=[])
                        nop.engine = ins.engine
                        nop.sync_info = mybir.SyncInfo(on_wait=[], on_update=c)
                        insts.insert(i + 1, nop)
                        i += 1
                i += 1


def _build_program(need_bq, j0, fix=True):
    import concourse.bass as bass
    import concourse.mybir as mybir
    from concourse.tile import TileContext

    F32 = mybir.dt.float32
    BF16 = mybir.dt.bfloat16
    FP8 = mybir.dt.float8e4
    AF = mybir.ActivationFunctionType
    OP = mybir.AluOpType

    nc = bass.Bass()

    # ---- DRAM I/O ----
    XT = nc.dram_tensor("XT", [BLOC, D, L], BF16, kind="ExternalInput")
    XBF = nc.dram_tensor("XBF", [BLOC, 128, NLC * D], BF16, kind="ExternalInput")
    FCT = nc.dram_tensor("FCT", [128, NLC * 128], BF16, kind="ExternalInput")
    C2S2 = nc.dram_tensor("C2S2", [128, L], BF16, kind="ExternalInput")
    WQT = nc.dram_tensor("WQT", [D, D], BF16, kind="ExternalInput")
    WOT = nc.dram_tensor("WOT", [D, D], BF16, kind="ExternalInput")
    WPK = nc.dram_tensor("WPK", [H, 128, M * 128], FP8, kind="ExternalInput")
    W1T = nc.dram_tensor("W1T", [D, DFF], BF16, kind="ExternalInput")
    W2T = nc.dram_tensor("W2T", [DFF, D], BF16, kind="ExternalInput")
    EYE = nc.dram_tensor("EYE", [128, 128], BF16, kind="ExternalInput")
    BO4 = nc.dram_tensor("BO4", [128, NDC], F32, kind="ExternalInput")
    BQ4 = nc.dram_tensor("BQ4", [128, NDC], F32, kind="ExternalInput")
    DECS = nc.dram_tensor("DECS", [128, 4], F32, kind="ExternalInput")
    OUT_T = nc.dram_tensor("OUT_T", [BLOC, D, L], BF16, kind="ExternalOutput")

    with TileContext(nc) as tc:
        # ---------- persistent pools (LIFO close at the end) ----------
        cst = tc.tile_pool(name="cst", bufs=1)
        cstp = cst.__enter__()

        # small constants first: DFT can start as soon as fct + xbf land
        fct = cstp.tile([128, NLC * 128], BF16, name="fct")
        nc.sync.dma_start(out=fct[:], in_=FCT[:])
        wqt = [cstp.tile([128, D], BF16, name=f"wqt{i}") for i in range(NDC)]
        for i in range(NDC):
            nc.sync.dma_start(out=wqt[i][:], in_=WQT[i * 128:(i + 1) * 128, :])
        c2s2 = cstp.tile([128, L], BF16, name="c2s2")
        nc.sync.dma_start(out=c2s2[:], in_=C2S2[:])
        wot = [cstp.tile([128, D], BF16, name=f"wot{i}") for i in range(NDC)]
        for i in range(NDC):
            nc.sync.dma_start(out=wot[i][:], in_=WOT[i * 128:(i + 1) * 128, :])
        eye = cstp.tile([128, 128], BF16, name="eye")
        nc.sync.dma_start(out=eye[:], in_=EYE[:])
        bo4 = cstp.tile([128, NDC], F32, name="bo4")
        nc.sync.dma_start(out=bo4[:], in_=BO4[:])
        decs = cstp.tile([128, 4], F32, name="decs")
        nc.sync.dma_start(out=decs[:], in_=DECS[:])
        bq4 = None
        if need_bq:
            bq4 = cstp.tile([128, NDC], F32, name="bq4")
            nc.sync.dma_start(out=bq4[:], in_=BQ4[:])
        ones12 = cstp.tile([128, PAD], BF16, name="ones12")
        nc.vector.memset(ones12[:], 1.0)

        main = tc.tile_pool(name="main", bufs=1)
        mainp = main.__enter__()
        # x^T anchor (bf16) and u/z value buffer (bf16, written in place)
        mt = [[mainp.tile([128, L], BF16, name=f"m_{b}_{dc}") for dc in range(NDC)]
              for b in range(BLOC)]
        uz = [[mainp.tile([128, L], BF16, name=f"uz_{b}_{dc}") for dc in range(NDC)]
              for b in range(BLOC)]
        # decomp1 output (FFN input), bf16
        r1 = [[mainp.tile([128, L], BF16, name=f"r1_{b}_{dc}") for dc in range(NDC)]
              for b in range(BLOC)]

        decp_cm = tc.tile_pool(name="decp", bufs=2)
        decp = decp_cm.__enter__()

        # ---------- scan-free chunked series decomposition ----------
        def gate_chunk(b, dc, c, dw_col, db_col):
            src = uz[b][dc]
            c0 = c * CH
            g = decp.tile([128, CH], BF16, name=f"g{dc}", tag=f"g{dc}")
            nc.scalar.activation(g[:], src[:, c0:c0 + CH], AF.Sigmoid,
                                 scale=decs[:, dw_col:dw_col + 1],
                                 bias=decs[:, db_col:db_col + 1])
            return g

        def decomp_chunk(b, dc, c, g, dst, par):
            """src = uz[b][dc] bf16; writes residual chunk c to dst slice."""
            src = uz[b][dc]
            c0 = c * CH
            if c == 0 or c == NTC - 1:
                up = decp.tile([128, UPW], BF16, name="up", tag="up")
                bcol = decp.tile([128, 1], F32, name="bcol", tag="bcol")
                if c == 0:
                    nc.vector.tensor_copy(bcol[:], src[:, 0:1])
                    nc.vector.tensor_copy(up[:, PAD:UPW], src[:, 0:CH + PAD])
                    nc.vector.tensor_scalar_mul(up[:, 0:PAD], ones12[:], bcol[:])
                else:
                    nc.vector.tensor_copy(bcol[:], src[:, L - 1:L])
                    nc.vector.tensor_copy(up[:, 0:UPW - PAD], src[:, c0 - PAD:L])
                    nc.vector.tensor_scalar_mul(up[:, UPW - PAD:UPW], ones12[:],
                                                bcol[:])
                upv = up
                u0 = 0
            else:
                upv = src
                u0 = c0 - PAD
            # windowed sums via shift-adds (all bf16)
            s2 = decp.tile([128, 535], BF16, name="s2", tag="s2")
            nc.gpsimd.tensor_tensor(s2[:], upv[:, u0:u0 + 535],
                                    upv[:, u0 + 1:u0 + 536], OP.add)
            s4 = decp.tile([128, 533], BF16, name="s4", tag="s4")
            e4 = nc.gpsimd if par else nc.vector
            e4.tensor_tensor(s4[:], s2[:, 0:533], s2[:, 2:535], OP.add)
            s8 = decp.tile([128, 529], BF16, name="s8", tag="s8")
            nc.vector.tensor_tensor(s8[:], s4[:, 0:529], s4[:, 4:533], OP.add)
            s12 = decp.tile([128, 525], BF16, name="s12", tag="s12")
            nc.vector.tensor_tensor(s12[:], s8[:, 0:525], s4[:, 8:533], OP.add)
            s13 = decp.tile([128, 524], BF16, name="s13", tag="s13")
            nc.vector.tensor_tensor(s13[:], s12[:, 0:524],
                                    upv[:, u0 + 12:u0 + 536], OP.add)
            s25 = decp.tile([128, CH], BF16, name="s25", tag="s25")
            nc.vector.tensor_tensor(s25[:], s12[:, 0:CH], s13[:, 12:12 + CH], OP.add)
            ma = decp.tile([128, CH], BF16, name="ma", tag="ma")
            nc.vector.tensor_scalar_mul(ma[:], s25[:], 1.0 / 25.0)
            dl = decp.tile([128, CH], BF16, name="dl", tag="dl")
            nc.vector.scalar_tensor_tensor(dl[:], s13[:, 6:6 + CH], 1.0 / 13.0,
                                           ma[:], OP.mult, OP.subtract)
            e = decp.tile([128, CH], BF16, name="e", tag="e")
            nc.gpsimd.tensor_tensor(e[:], g[:], dl[:], OP.mult)
            t1 = decp.tile([128, CH], BF16, name="t1", tag="dl")
            nc.vector.tensor_tensor(t1[:], ma[:], e[:], OP.add)
            er = nc.vector if par else nc.gpsimd
            er.tensor_tensor(dst, src[:, c0:c0 + CH], t1[:], OP.subtract)
            return None

        # ---------- Fourier front: DFT + q-projection (both batches) ----------
        fr_cm = tc.tile_pool(name="fr", bufs=1)
        fr = fr_cm.__enter__()
        frp_cm = tc.tile_pool(name="frp", bufs=2, space="PSUM")
        frp = frp_cm.__enter__()
        xbf_cm = tc.tile_pool(name="xbfp", bufs=1)
        xbfp = xbf_cm.__enter__()

        qt = [[None] * NDC for _ in range(BLOC)]
        for b in range(BLOC):
            xbf = xbfp.tile([128, NLC * D], BF16, name=f"xbf{b}", tag="xbf")
            for gch in range(4):
                nc.sync.dma_start(out=xbf[:, gch * 4 * D:(gch + 1) * 4 * D],
                                  in_=XBF[b][:, gch * 4 * D:(gch + 1) * 4 * D])
            psA4 = frp.tile([128, 512], F32, name="psA4", tag="fps")
            xselT = fr.tile([128, NDC * 128], BF16, name=f"xselT{b}", tag=f"xselT{b}")
            for gch in range(4):
                for dc in range(NDC):
                    for li in range(4):
                        lc = gch * 4 + li
                        nc.tensor.matmul(
                            psA4[:, dc * 128:(dc + 1) * 128],
                            xbf[:, lc * D + dc * 128: lc * D + (dc + 1) * 128],
                            fct[:, lc * 128:(lc + 1) * 128],
                            start=(lc == 0), stop=(lc == NLC - 1),
                            skip_group_check=True)
            for dc in range(NDC):
                nc.scalar.activation(xselT[:, dc * 128:(dc + 1) * 128],
                                     psA4[:, dc * 128:(dc + 1) * 128], AF.Copy)
            # q-projection in mode space
            for do in range(NDC):
                qt[b][do] = fr.tile([128, 128], BF16, name=f"qt{b}_{do}",
                                    tag=f"qt{b}_{do}")
                ps = frp.tile([128, 128], F32, name="psQ", tag="fps")
                for dc in range(NDC):
                    nc.tensor.matmul(
                        ps[:], wqt[dc][:, do * 128:(do + 1) * 128],
                        xselT[:, dc * 128:(dc + 1) * 128],
                        start=(dc == 0), stop=(dc == NDC - 1))
                if need_bq:
                    nc.vector.tensor_tensor(
                        ps[:, j0:j0 + 1], ps[:, j0:j0 + 1],
                        bq4[:, do:do + 1], OP.add)
                nc.scalar.activation(qt[b][do][:], ps[:], AF.Copy)
        xbf_cm.__exit__(None, None, None)

        # ---------- mode mix (both batches packed into columns; fp8) ----------
        wpk_cm = tc.tile_pool(name="wpkp", bufs=3)
        wpkp = wpk_cm.__enter__()
        rh = [fr.tile([128, 128], FP8, name=f"rh{h}", tag=f"rh{h}")
              for h in range(H)]
        for h in range(H):
            src_do, r0 = h // 2, (h % 2) * 64
            for b in range(BLOC):
                rhv = rh[h].rearrange("p (m t) -> p m t", t=2)
                nc.scalar.activation(rhv[0:64, :, b],
                                     qt[b][src_do][r0:r0 + 64, 0:64],
                                     AF.Copy, scale=0.25)
                nc.scalar.activation(rhv[64:128, :, b],
                                     qt[b][src_do][r0:r0 + 64, 64:128],
                                     AF.Copy, scale=0.25)
        otre = [[fr.tile([128, M], BF16, name=f"otre{b}_{dc}", tag=f"otre{b}{dc}")
                 for dc in range(NDC)] for b in range(BLOC)]
        otim = [[fr.tile([128, M], BF16, name=f"otim{b}_{dc}", tag=f"otim{b}{dc}")
                 for dc in range(NDC)] for b in range(BLOC)]
        for h in range(H):
            psm = frp.tile([128, 128], F32, name="psM", tag="fps")
            for q in range(2):
                wpk_q = wpkp.tile([128, 32 * 128], FP8, name=f"wpk{h}_{q}",
                                  tag="wpk")
                nc.sync.dma_start(out=wpk_q[:],
                                  in_=WPK[h][:, q * 4096:(q + 1) * 4096])
                for mq in range(32):
                    m = q * 32 + mq
                    nc.tensor.matmul(
                        psm[:, 2 * m:2 * m + 2],
                        wpk_q[:, mq * 128:(mq + 1) * 128],
                        rh[h][:, 2 * m:2 * m + 2],
                        start=True, stop=True)
            psv = psm.rearrange("p (m t) -> p m t", t=2)
            dc, r0 = h // 2, (h % 2) * 64
            for b in range(BLOC):
                nc.scalar.activation(otre[b][dc][r0:r0 + 64, :], psv[0:64, :, b],
                                     AF.Copy)
                nc.scalar.activation(otim[b][dc][r0:r0 + 64, :], psv[64:128, :, b],
                                     AF.Copy)
        wpk_cm.__exit__(None, None, None)

        # FFN weights arrive during the mode-mix / decomp1 window
        ffnw = tc.tile_pool(name="ffnw", bufs=1)
        ffnwp = ffnw.__enter__()
        w1t = [ffnwp.tile([128, DFF], BF16, name=f"w1t{i}") for i in range(NDC)]
        for i in range(NDC):
            nc.sync.dma_start(out=w1t[i][:], in_=W1T[i * 128:(i + 1) * 128, :])
        w2t = [ffnwp.tile([128, D], BF16, name=f"w2t{i}") for i in range(NFF)]
        for i in range(NFF):
            nc.sync.dma_start(out=w2t[i][:], in_=W2T[i * 128:(i + 1) * 128, :])
        # x^T arrives late (only needed at the iDFT residual add)
        for b in range(BLOC):
            for dc in range(NDC):
                nc.sync.dma_start(out=mt[b][dc][:],
                                  in_=XT[b, dc * 128:(dc + 1) * 128, :])

        # ---------- per-batch pipeline: Wo -> iDFT -> d1 -> FFN -> d2 ----------
        psy_cm = tc.tile_pool(name="psy", bufs=1, space="PSUM")
        psyp = psy_cm.__enter__()
        gq_cm = tc.tile_pool(name="gqp", bufs=16)
        gqp = gq_cm.__enter__()
        psh_cm = tc.tile_pool(name="pshp", bufs=2, space="PSUM")
        pshp = psh_cm.__enter__()
        psf_cm = tc.tile_pool(name="psfp", bufs=2, space="PSUM")
        psfp = psf_cm.__enter__()

        def wo_proj(b):
            pcat = fr.tile([128, D], BF16, name=f"pcat{b}", tag=f"pcat{b}")
            for ro, ot in ((0, otre[b]), (64, otim[b])):
                for do in range(NDC):
                    ps = frp.tile([128, M], F32, name="psP", tag="fps")
                    for dc in range(NDC):
                        nc.tensor.matmul(
                            ps[:], wot[dc][:, do * 128:(do + 1) * 128],
                            ot[dc][:], start=(dc == 0), stop=(dc == NDC - 1))
                    pp = fr.tile([128, M], BF16, name=f"pp{ro}_{do}", tag="pp")
                    nc.scalar.activation(pp[:], ps[:], AF.Copy)
                    pst = frp.tile([M, 128], BF16, name="psT", tag="fps")
                    nc.tensor.transpose(pst[:], pp[:], eye[:])
                    nc.scalar.activation(
                        pcat[ro:ro + 64, do * 128:(do + 1) * 128], pst[:], AF.Copy)
            return pcat

        def idft_chunk(b, pcat, c):
            for dc in range(NDC):
                psy = psyp.tile([128, CH], F32, name="psY", tag="psY")
                nc.tensor.matmul(
                    psy[:], pcat[:, dc * 128:(dc + 1) * 128],
                    c2s2[:, c * CH:(c + 1) * CH],
                    start=True, stop=True)
                nc.vector.scalar_tensor_tensor(
                    uz[b][dc][:, c * CH:(c + 1) * CH], psy[:], bo4[:, dc:dc + 1],
                    mt[b][dc][:, c * CH:(c + 1) * CH], OP.add, OP.add)

        def ffn_chunk(b, c):
            gqs = []
            for ff in range(NFF):
                psh = pshp.tile([128, CH], F32, name="psH", tag="psH")
                for dc in range(NDC):
                    nc.tensor.matmul(
                        psh[:], w1t[dc][:, ff * 128:(ff + 1) * 128],
                        r1[b][dc][:, c * CH:(c + 1) * CH],
                        start=(dc == 0), stop=(dc == NDC - 1))
                gqv = gqp.tile([128, CH], BF16, name="gq", tag="gq")
                nc.scalar.activation(gqv[:], psh[:], AF.Gelu)
                gqs.append(gqv)
            for do in range(NDC):
                psf = psfp.tile([128, CH], F32, name="psF", tag="psF")
                for ff in range(NFF):
                    nc.tensor.matmul(
                        psf[:], w2t[ff][:, do * 128:(do + 1) * 128],
                        gqs[ff][:], start=(ff == 0), stop=(ff == NFF - 1))
                nc.vector.tensor_tensor(uz[b][do][:, c * CH:(c + 1) * CH], psf[:],
                                        r1[b][do][:, c * CH:(c + 1) * CH], OP.add)

        def d1_chunk(b, c, gs):
            for dc in range(NDC):
                decomp_chunk(b, dc, c, gs[c][dc],
                             r1[b][dc][:, c * CH:(c + 1) * CH], (dc + c) % 2)

        def d2_chunk(b, c, gs):
            for dc in range(NDC):
                ob = decp.tile([128, CH], BF16, name="ob", tag="ob")
                decomp_chunk(b, dc, c, gs[c][dc], ob[:], (dc + c) % 2)
                nc.sync.dma_start(
                    out=OUT_T[b, dc * 128:(dc + 1) * 128, c * CH:(c + 1) * CH],
                    in_=ob[:])

        for b in range(BLOC):
            g1 = [None] * NTC
            g2 = [None] * NTC
            pcat = wo_proj(b)
            idft_chunk(b, pcat, 0)
            idft_chunk(b, pcat, 1)
            g1[0] = [gate_chunk(b, dc, 0, 0, 1) for dc in range(NDC)]
            g1[1] = [gate_chunk(b, dc, 1, 0, 1) for dc in range(NDC)]
            d1_chunk(b, 0, g1)
            idft_chunk(b, pcat, 2)
            d1_chunk(b, 1, g1)
            ffn_chunk(b, 0)
            idft_chunk(b, pcat, 3)
            g1[2] = [gate_chunk(b, dc, 2, 0, 1) for dc in range(NDC)]
            g1[3] = [gate_chunk(b, dc, 3, 0, 1) for dc in range(NDC)]
            d1_chunk(b, 2, g1)
            ffn_chunk(b, 1)
            d1_chunk(b, 3, g1)
            g2[0] = [gate_chunk(b, dc, 0, 2, 3) for dc in range(NDC)]
            g2[1] = [gate_chunk(b, dc, 1, 2, 3) for dc in range(NDC)]
            d2_chunk(b, 0, g2)
            ffn_chunk(b, 2)
            d2_chunk(b, 1, g2)
            ffn_chunk(b, 3)
            g2[2] = [gate_chunk(b, dc, 2, 2, 3) for dc in range(NDC)]
            g2[3] = [gate_chunk(b, dc, 3, 2, 3) for dc in range(NDC)]
            d2_chunk(b, 2, g2)
            d2_chunk(b, 3, g2)

        psf_cm.__exit__(None, None, None)
        psh_cm.__exit__(None, None, None)
        gq_cm.__exit__(None, None, None)
        psy_cm.__exit__(None, None, None)
        ffnw.__exit__(None, None, None)
        frp_cm.__exit__(None, None, None)
        fr_cm.__exit__(None, None, None)
        decp_cm.__exit__(None, None, None)
        main.__exit__(None, None, None)
        cst.__exit__(None, None, None)

    if fix:
        _fix_sync_waits(nc)
    return nc


def _host_prep(inputs):
    import ml_dtypes
    bf16 = ml_dtypes.bfloat16
    x = np.asarray(inputs["x"], np.float32)
    modes = np.asarray(inputs["mode_index"]).astype(np.int64)
    l = np.arange(L, dtype=np.float64)
    ang = 2.0 * np.pi * np.outer(l, modes.astype(np.float64)) / L
    FC = np.concatenate([np.cos(ang), -np.sin(ang)], axis=1)          # [L, 128]
    m_out = np.arange(M, dtype=np.float64)
    w = np.where(m_out == 0, 1.0, 2.0) / L
    ang2 = 2.0 * np.pi * np.outer(m_out, l) / L
    C2 = np.concatenate([w[:, None] * np.cos(ang2),
                         w[:, None] * -np.sin(ang2)], axis=0)         # [128, L]
    C2 *= 4.0 / 2.0 ** 18    # undo the fp8 wpk (2^18) and rh (1/4) pre-scales

    FCT = FC.reshape(NLC, 128, 128).transpose(1, 0, 2).reshape(128, NLC * 128)

    wr = np.asarray(inputs["four_wr"], np.float64)   # [H, E, O, M]
    wi = np.asarray(inputs["four_wi"], np.float64)
    wpk = np.zeros((H, M, 128, 128), np.float64)
    wpk[:, :, 0:64, 0:64] = wr.transpose(0, 3, 1, 2)
    wpk[:, :, 0:64, 64:128] = wi.transpose(0, 3, 1, 2)
    wpk[:, :, 64:128, 0:64] = -wi.transpose(0, 3, 1, 2)
    wpk[:, :, 64:128, 64:128] = wr.transpose(0, 3, 1, 2)
    WPKh = wpk.transpose(0, 2, 1, 3).reshape(H, 128, M * 128)

    dec1_w = np.asarray(inputs["dec1_w"], np.float64)
    dec1_b = np.asarray(inputs["dec1_b"], np.float64)
    dec2_w = np.asarray(inputs["dec2_w"], np.float64)
    dec2_b = np.asarray(inputs["dec2_b"], np.float64)
    decs = np.zeros((128, 4), np.float32)
    decs[:, 0] = dec1_w[0] - dec1_w[1]
    decs[:, 1] = dec1_b[0] - dec1_b[1]
    decs[:, 2] = dec2_w[0] - dec2_w[1]
    decs[:, 3] = dec2_b[0] - dec2_b[1]

    bo = np.asarray(inputs["bo"], np.float32)
    bq = np.asarray(inputs["bq"], np.float32)
    BO4 = np.ascontiguousarray(bo.reshape(NDC, 128).T).astype(np.float32)
    zero_pos = np.nonzero(modes == 0)[0]
    need_bq = bool(len(zero_pos)) and bool(np.any(bq != 0))
    j0 = int(zero_pos[0]) if need_bq else 0
    BQ4 = np.ascontiguousarray((L * bq).reshape(NDC, 128).T).astype(np.float32)

    shared = {
        "FCT": FCT.astype(bf16),
        "C2S2": C2.astype(bf16),
        "WQT": np.ascontiguousarray(np.asarray(inputs["Wq"], np.float32).T).astype(bf16),
        "WOT": np.ascontiguousarray(np.asarray(inputs["Wo"], np.float32).T).astype(bf16),
        "WPK": (WPKh * 2.0 ** 18).astype(ml_dtypes.float8_e4m3),
        "W1T": np.ascontiguousarray(np.asarray(inputs["conv1_w"], np.float32).T).astype(bf16),
        "W2T": np.ascontiguousarray(np.asarray(inputs["conv2_w"], np.float32).T).astype(bf16),
        "EYE": np.eye(128, dtype=np.float32).astype(bf16),
        "BO4": BO4, "BQ4": BQ4,
        "DECS": decs,
    }
    in_maps = []
    for c in range(NC_):
        xl = x[c * BLOC:(c + 1) * BLOC]                       # [2, L, D]
        XTc = np.ascontiguousarray(xl.transpose(0, 2, 1))     # [2, D, L]
        xbf = xl.astype(bf16)                                 # [2, L, D]
        XBFc = np.ascontiguousarray(
            xbf.reshape(BLOC, NLC, 128, D).transpose(0, 2, 1, 3)
        ).reshape(BLOC, 128, NLC * D)
        im = dict(shared)
        im["XT"] = XTc.astype(bf16)
        im["XBF"] = XBFc
        in_maps.append(im)
    return in_maps, need_bq, j0


def kernel(**inputs):
    from concourse.bass_utils import run_bass_kernel_spmd

    in_maps, need_bq, j0 = _host_prep(inputs)
    key = (need_bq, j0)
    if key not in _prog_cache:
        _prog_cache[key] = _build_program(need_bq, j0)
    nc = _prog_cache[key]
    res = run_bass_kernel_spmd(nc, in_maps, core_ids=list(range(NC_)))
    outs = []
    for c in range(NC_):
        ot = np.asarray(res.results[c]["OUT_T"]).astype(np.float32)  # [2, D, L]
        outs.append(np.ascontiguousarray(ot.transpose(0, 2, 1)))
    return np.concatenate(outs, axis=0).astype(np.float32)
